# revision 5
# baseline (speedup 1.0000x reference)
"""Causal self-attention (B=4, T=2048, C=1024, H=16) on 8 Trainium2 cores.

Sharding: core c -> batch b = c//2, head-group g = c%2 (8 heads each,
tensor-parallel). QKV + attention + c_proj computed per core on its head
slice; partial c_proj outputs of a (b) pair are summed with an on-device
ReduceScatter over the T dimension; host concatenates the halves.

Self-contained: only imports concourse (installed library) + numpy.
"""

import ml_dtypes
import numpy as np

import concourse.bass as bass
import concourse.mybir as mybir
import concourse.tile as tile
from concourse import bacc
from concourse.bass_utils import run_bass_kernel_spmd
from concourse.masks import make_identity

B, T, C = 4, 2048, 1024
H_TOTAL, D = 16, 64
N_CORES = 8
HL = H_TOTAL // 2  # local heads per core (8)
HC = HL * D  # local head cols (512)
P = 128
TT = T // P  # 16 t-chunks of 128
CK = C // P  # 8 contraction chunks for qkv
F32 = mybir.dt.float32
F32R = mybir.dt.float32r
BF16 = mybir.dt.bfloat16
MASK_VAL = -480.0  # -60 after the 1/8 attention scale; exp(-60) ~ 0
SCALE = 1.0 / 8.0  # 1/sqrt(D)

_CACHE = {}


def _build_nc():
    nc = bacc.Bacc("TRN2", target_bir_lowering=False, debug=False, num_devices=N_CORES)

    x_d = nc.dram_tensor("x", [T, C], F32, kind="ExternalInput")
    wq_d = nc.dram_tensor("wq", [C, HC], F32R, kind="ExternalInput")
    wk_d = nc.dram_tensor("wk", [C, HC], F32R, kind="ExternalInput")
    wv_d = nc.dram_tensor("wv", [C, HC], F32R, kind="ExternalInput")
    bq_d = nc.dram_tensor("bq", [P, HC // P], F32, kind="ExternalInput")
    bk_d = nc.dram_tensor("bk", [P, HC // P], F32, kind="ExternalInput")
    bv_d = nc.dram_tensor("bv", [P, HC], F32, kind="ExternalInput")
    wp_d = nc.dram_tensor("wp", [HC, C], BF16, kind="ExternalInput")
    bp_d = nc.dram_tensor("bp", [P, C], F32, kind="ExternalInput")
    out_d = nc.dram_tensor("out", [T // 2, C], F32, kind="ExternalOutput")

    wq_r = wq_d.rearrange("(ko ki) n -> ki ko n", ki=P)
    wk_r = wk_d.rearrange("(ko ki) n -> ki ko n", ki=P)
    wv_r = wv_d.rearrange("(ko ki) n -> ki ko n", ki=P)
    wp_r = wp_d.rearrange("(ko ki) n -> ki ko n", ki=P)

    with tile.TileContext(nc) as tc:
        with (
            tc.tile_pool(name="const", bufs=1) as constp,
            tc.tile_pool(name="big", bufs=1) as bigp,
            tc.tile_pool(name="rot", bufs=2) as rotp,
            tc.tile_pool(name="xin", bufs=2) as xinp,
            tc.tile_pool(name="wqk", bufs=2) as wqkp,
            tc.tile_pool(name="wbig", bufs=1) as wbigp,
            tc.tile_pool(name="work", bufs=4) as workp,
            tc.tile_pool(name="zout", bufs=2) as zoutp,
            tc.tile_pool(name="score_ps", bufs=2, space="PSUM") as score_ps,
            tc.tile_pool(name="small_ps", bufs=2, space="PSUM") as small_ps,
            tc.tile_pool(name="mm_ps", bufs=2, space="PSUM") as mm_ps,
            tc.tile_pool(name="dram", bufs=1, space="DRAM") as dramp,
        ):
            # ---- constants ----
            ident = constp.tile([P, P], F32)
            make_identity(nc, ident)
            ident_bf = constp.tile([P, P], BF16)
            nc.vector.tensor_copy(out=ident_bf[:], in_=ident[:])
            # additive causal mask for the diagonal 128x128 block:
            # mask[s, u] = 0 where u >= s else MASK_VAL
            dmask = constp.tile([P, P], F32)
            nc.gpsimd.memset(dmask, 0.0)
            nc.gpsimd.affine_select(
                out=dmask,
                in_=dmask,
                compare_op=mybir.AluOpType.is_ge,
                fill=MASK_VAL,
                base=0,
                pattern=[[1, P]],
                channel_multiplier=-1,
            )
            bq_sb = constp.tile([P, HC // P], F32)
            nc.sync.dma_start(bq_sb[:], bq_d[:])
            bk_sb = constp.tile([P, HC // P], F32)
            nc.sync.dma_start(bk_sb[:], bk_d[:])
            bv_sb = constp.tile([P, HC], F32)
            nc.sync.dma_start(bv_sb[:], bv_d[:])

            # ---- persistent activations ----
            qT = bigp.tile([P, HC // P, T], F32R)  # q^T  [qcol, t]
            kT = bigp.tile([P, HC // P, T], F32R)  # k^T  [kcol, t]
            v_ext = bigp.tile([P, TT, HL, D + 1], BF16)  # v with ones col
            y_sb = bigp.tile([P, TT, HC], BF16)  # y    [t, ci]
            # rotating 32KB slots: xT halves -> p buffers -> yT
            xT_a = rotp.tile([P, CK // 2, T], F32R, tag="rot")
            xT_b = rotp.tile([P, CK // 2, T], F32R, tag="rot")

            def xT(ck):
                return xT_a[:, ck] if ck < CK // 2 else xT_b[:, ck - CK // 2]

            nc.vector.memset(v_ext[:, :, :, D : D + 1], 1.0)

            # ---- transpose x: [T, C] -> xT [C, T] ----
            for tt in range(TT):
                xin = xinp.tile([P, C], F32, tag="xin")
                nc.sync.dma_start(xin[:], x_d[tt * P : (tt + 1) * P, :])
                for c in range(CK):
                    ps = small_ps.tile([P, P], F32, tag="sp")
                    nc.tensor.transpose(ps[:], xin[:, c * P : (c + 1) * P], ident[:])
                    nc.vector.tensor_copy(
                        out=xT(c)[:, tt * P : (tt + 1) * P], in_=ps[:]
                    )

            # ---- q^T, k^T = (w^T x^T) + bias ----
            for w_r, b_sb, dstT in ((wq_r, bq_sb, qT), (wk_r, bk_sb, kT)):
                for j in range(HC // P):
                    wj = wqkp.tile([P, CK, P], F32R, tag="wqk")
                    nc.sync.dma_start(wj[:], w_r[:, :, j * P : (j + 1) * P])
                    for u in range(T // 512):
                        ps = mm_ps.tile([P, 512], F32, tag="mm")
                        for ck in range(CK):
                            nc.tensor.matmul(
                                ps[:],
                                wj[:, ck, :],
                                xT(ck)[:, u * 512 : (u + 1) * 512],
                                start=(ck == 0),
                                stop=(ck == CK - 1),
                            )
                        nc.vector.tensor_add(
                            out=dstT[:, j, u * 512 : (u + 1) * 512],
                            in0=ps[:],
                            in1=b_sb[:, j : j + 1].to_broadcast((P, 512)),
                        )

            # ---- v = (x w_v) + bias, bf16, with ones column ----
            wv_sb = wbigp.tile([P, CK, HC], F32R, tag="wbig")
            nc.sync.dma_start(wv_sb[:], wv_r[:])
            for tt in range(TT):
                ps = mm_ps.tile([P, 512], F32, tag="mm")
                for ck in range(CK):
                    nc.tensor.matmul(
                        ps[:],
                        xT(ck)[:, tt * P : (tt + 1) * P],
                        wv_sb[:, ck, :],
                        start=(ck == 0),
                        stop=(ck == CK - 1),
                    )
                nc.vector.tensor_add(
                    out=v_ext[:, tt, :, 0:D],
                    in0=ps[:].rearrange("p (h d) -> p h d", d=D),
                    in1=bv_sb[:].rearrange("p (h d) -> p h d", d=D),
                )

            # ---- attention per (head, t-half) ----
            for h in range(HL):
                hj = h // 2  # 128-col chunk in qT/kT
                hb = (h % 2) * D  # base partition within the chunk
                for u in range(2):  # t in [1024u, 1024u+1024)
                    n_i = 8 * (u + 1)  # s-chunks needed for this half
                    p_sb = rotp.tile([P, TT, 1024], BF16, tag="rot")
                    for i in range(n_i):
                        ps = score_ps.tile([P, 1024], F32, tag="score")
                        for j in range(2 * u, 2 * u + 2):
                            if j < i // 4:
                                continue
                            c0 = (j - 2 * u) * 512
                            nc.tensor.matmul(
                                ps[:, c0 : c0 + 512],
                                kT[hb : hb + D, hj, i * P : (i + 1) * P],
                                qT[
                                    hb : hb + D, hj, j * 512 : (j + 1) * 512
                                ],
                                start=True,
                                stop=True,
                            )
                        if i // 8 == u:
                            d0 = i * P - 1024 * u
                            nc.vector.tensor_add(
                                out=ps[:, d0 : d0 + P],
                                in0=ps[:, d0 : d0 + P],
                                in1=dmask[:],
                            )
                        c0 = max(0, i * P - 1024 * u)
                        nc.scalar.activation(
                            out=p_sb[:, i, c0:1024],
                            in_=ps[:, c0:1024],
                            func=mybir.ActivationFunctionType.Exp,
                            scale=SCALE,
                        )
                    for tl in range(8):
                        t_chunk = 8 * u + tl
                        ps_av = small_ps.tile([P, D + 1], F32, tag="sp")
                        for i in range(t_chunk + 1):
                            nc.tensor.matmul(
                                ps_av[:],
                                p_sb[:, i, tl * P : (tl + 1) * P],
                                v_ext[:, i, h, :],
                                start=(i == 0),
                                stop=(i == t_chunk),
                            )
                        recip = workp.tile([P, 1], F32, tag="recip")
                        nc.vector.reciprocal(recip[:], ps_av[:, D : D + 1])
                        nc.vector.tensor_mul(
                            out=y_sb[:, t_chunk, h * D : (h + 1) * D],
                            in0=ps_av[:, 0:D],
                            in1=recip[:, 0:1].to_broadcast((P, D)),
                        )

            # ---- transpose y -> yT (bf16) ----
            yT = rotp.tile([P, HC // P, T], BF16, tag="rot")
            for c in range(HC // P):
                for tt in range(TT):
                    ps = small_ps.tile([P, P], BF16, tag="sp")
                    nc.tensor.transpose(
                        ps[:], y_sb[:, tt, c * P : (c + 1) * P], ident_bf[:]
                    )
                    nc.vector.tensor_copy(
                        out=yT[:, c, tt * P : (tt + 1) * P], in_=ps[:]
                    )

            # ---- c_proj + bias -> z_dram; pair ReduceScatter over T ----
            wp_sb = wbigp.tile([P, HC // P, C], BF16, tag="wbig")
            nc.sync.dma_start(wp_sb[:], wp_r[:])
            bp_sb = constp.tile([P, C], F32)
            nc.sync.dma_start(bp_sb[:], bp_d[:])
            z_dram = dramp.tile([T, C], F32)
            rs_out = dramp.tile([T // 2, C], F32)
            for tt in range(TT):
                for n in range(C // 512):
                    ps = mm_ps.tile([P, 512], F32, tag="mm")
                    for c in range(HC // P):
                        nc.tensor.matmul(
                            ps[:],
                            yT[:, c, tt * P : (tt + 1) * P],
                            wp_sb[:, c, n * 512 : (n + 1) * 512],
                            start=(c == 0),
                            stop=(c == HC // P - 1),
                        )
                    z_sb = zoutp.tile([P, 512], F32, tag="z")
                    nc.vector.tensor_add(
                        out=z_sb[:], in0=ps[:], in1=bp_sb[:, n * 512 : (n + 1) * 512]
                    )
                    nc.sync.dma_start(
                        z_dram[tt * P : (tt + 1) * P, n * 512 : (n + 1) * 512],
                        z_sb[:],
                    )
            nc.gpsimd.collective_compute(
                "ReduceScatter",
                mybir.AluOpType.add,
                replica_groups=[[0, 1], [2, 3], [4, 5], [6, 7]],
                ins=[z_dram.opt()],
                outs=[rs_out.opt()],
            )
            nc.sync.dma_start(out_d[:], rs_out[:])

    nc.compile()
    return nc


def _in_maps(inputs):
    x = np.ascontiguousarray(inputs["x"], dtype=np.float32)
    w_attn = np.asarray(inputs["w_attn"], dtype=np.float32)
    b_attn = np.asarray(inputs["b_attn"], dtype=np.float32)
    w_proj = np.asarray(inputs["w_proj"], dtype=np.float32)
    b_proj = np.asarray(inputs["b_proj"], dtype=np.float32)

    maps = []
    for c in range(N_CORES):
        b, g = c // 2, c % 2
        s = g * HC
        bq = b_attn[s : s + HC].reshape(HC // P, P).T
        bk = b_attn[C + s : C + s + HC].reshape(HC // P, P).T
        bv = np.broadcast_to(b_attn[2 * C + s : 2 * C + s + HC], (P, HC))
        bp = (
            np.broadcast_to(b_proj, (P, C))
            if g == 0
            else np.zeros((P, C), np.float32)
        )
        maps.append(
            {
                "x": x[b],
                "wq": np.ascontiguousarray(w_attn[:, s : s + HC]),
                "wk": np.ascontiguousarray(w_attn[:, C + s : C + s + HC]),
                "wv": np.ascontiguousarray(w_attn[:, 2 * C + s : 2 * C + s + HC]),
                "bq": np.ascontiguousarray(bq),
                "bk": np.ascontiguousarray(bk),
                "bv": np.ascontiguousarray(bv),
                "wp": np.ascontiguousarray(
                    w_proj[s : s + HC, :].astype(ml_dtypes.bfloat16)
                ),
                "bp": np.ascontiguousarray(bp),
            }
        )
    return maps


def _run(inputs, trace=False, trace_cores=None):
    if "nc" not in _CACHE:
        _CACHE["nc"] = _build_nc()
    nc = _CACHE["nc"]
    res = run_bass_kernel_spmd(
        nc,
        _in_maps(inputs),
        list(range(N_CORES)),
        trace=trace,
        trace_cores=trace_cores,
    )
    out = np.empty((B, T, C), np.float32)
    for b in range(B):
        out[b, : T // 2] = res.results[2 * b]["out"]
        out[b, T // 2 :] = res.results[2 * b + 1]["out"]
    return out, res


def kernel(**inputs):
    out, _ = _run(inputs)
    return out


# revision 7
# speedup vs baseline: 1.2877x; 1.2877x over previous
"""Causal self-attention (B=4, T=2048, C=1024, H=16) on 8 Trainium2 cores.

Sharding: core c -> batch b = c//2, head-group g = c%2 (8 heads each,
tensor-parallel). QKV + attention + c_proj computed per core on its head
slice; partial c_proj outputs of a (b) pair are summed with chunked
on-device ReduceScatters over the T dimension; host reassembles.

Self-contained: only imports concourse (installed library) + numpy.
"""

import ml_dtypes
import numpy as np

import concourse.mybir as mybir
import concourse.tile as tile
from concourse import bacc
from concourse.bass_utils import run_bass_kernel_spmd
from concourse.masks import make_identity

B, T, C = 4, 2048, 1024
H_TOTAL, D = 16, 64
N_CORES = 8
HL = H_TOTAL // 2  # local heads per core (8)
HC = HL * D  # local head cols (512)
NP = HL // 2  # head pairs (4)
P = 128
TT = T // P  # 16 t-chunks of 128
CK = C // P  # 8 contraction chunks for qkv
RS_CHUNKS = 4
F32 = mybir.dt.float32
F32R = mybir.dt.float32r
BF16 = mybir.dt.bfloat16
MASK_VAL = -480.0  # -60 after the 1/8 attention scale; exp(-60) ~ 0
SCALE = 1.0 / 8.0  # 1/sqrt(D)

_CACHE = {}


def _build_nc():
    nc = bacc.Bacc("TRN2", target_bir_lowering=False, debug=False, num_devices=N_CORES)

    x_d = nc.dram_tensor("x", [T, C], F32, kind="ExternalInput")
    # weights pre-laid-out on host for contiguous DMA
    wq_d = nc.dram_tensor("wq", [P, NP, CK, P], F32R, kind="ExternalInput")
    wk_d = nc.dram_tensor("wk", [P, NP, CK, P], F32R, kind="ExternalInput")
    wv_d = nc.dram_tensor("wv", [P, CK, HC], F32R, kind="ExternalInput")
    bq_d = nc.dram_tensor("bq", [P, NP], F32, kind="ExternalInput")
    bk_d = nc.dram_tensor("bk", [P, NP], F32, kind="ExternalInput")
    bv_d = nc.dram_tensor("bv", [P, HC], F32, kind="ExternalInput")
    wp_d = nc.dram_tensor("wp", [P, HC // P, C], BF16, kind="ExternalInput")
    bp_d = nc.dram_tensor("bp", [P, C], F32, kind="ExternalInput")
    out_d = nc.dram_tensor("out", [T // 2, C], F32, kind="ExternalOutput")

    with tile.TileContext(nc) as tc:
        with (
            tc.tile_pool(name="const", bufs=1) as constp,
            tc.tile_pool(name="big", bufs=1) as bigp,
            tc.tile_pool(name="rot", bufs=3) as rotp,
            tc.tile_pool(name="xin", bufs=2) as xinp,
            tc.tile_pool(name="wqk", bufs=2) as wqkp,
            tc.tile_pool(name="ypair", bufs=4) as ypairp,
            tc.tile_pool(name="work", bufs=4) as workp,
            tc.tile_pool(name="zout", bufs=2) as zoutp,
            tc.tile_pool(name="score_ps", bufs=2, space="PSUM") as score_ps,
            tc.tile_pool(name="small_ps", bufs=2, space="PSUM") as small_ps,
            tc.tile_pool(name="mm_ps", bufs=2, space="PSUM") as mm_ps,
            tc.tile_pool(name="dram", bufs=1, space="DRAM") as dramp,
        ):
            # ---- constants ----
            ident = constp.tile([P, P], F32)
            make_identity(nc, ident)
            ident_bf = constp.tile([P, P], BF16)
            nc.vector.tensor_copy(out=ident_bf[:], in_=ident[:])
            # additive causal mask for the diagonal 128x128 block:
            # mask[s, u] = 0 where u >= s else MASK_VAL
            dmask = constp.tile([P, P], F32)
            nc.gpsimd.memset(dmask, 0.0)
            nc.gpsimd.affine_select(
                out=dmask,
                in_=dmask,
                compare_op=mybir.AluOpType.is_ge,
                fill=MASK_VAL,
                base=0,
                pattern=[[1, P]],
                channel_multiplier=-1,
            )
            bq_sb = constp.tile([P, NP], F32)
            nc.sync.dma_start(bq_sb[:], bq_d[:])
            bk_sb = constp.tile([P, NP], F32)
            nc.sync.dma_start(bk_sb[:], bk_d[:])
            bv_sb = constp.tile([P, HC], F32)
            nc.sync.dma_start(bv_sb[:], bv_d[:])
            bp_sb = constp.tile([P, C], F32)
            nc.sync.dma_start(bp_sb[:], bp_d[:])

            # ---- persistent activations ----
            qT = bigp.tile([P, NP, T], BF16)  # q^T [qcol, t]
            kT = bigp.tile([P, NP, T], BF16)  # k^T [kcol, t]
            v_ext = bigp.tile([P, TT, HL, D + 1], BF16)  # v with ones col
            yT = bigp.tile([P, NP, T], BF16)  # y^T [ci, t]
            # rotating 32KB slots: xT halves + wv -> p buffers -> wp
            xT_a = rotp.tile([P, CK // 2, T], F32R, tag="rot")
            xT_b = rotp.tile([P, CK // 2, T], F32R, tag="rot")

            def xT(ck):
                return xT_a[:, ck] if ck < CK // 2 else xT_b[:, ck - CK // 2]

            nc.vector.memset(v_ext[:, :, :, D : D + 1], 1.0)

            # ---- transpose x: [T, C] -> xT [C, T] ----
            for tt in range(TT):
                xin = xinp.tile([P, C], F32, tag="xin")
                nc.sync.dma_start(xin[:], x_d[tt * P : (tt + 1) * P, :])
                for c in range(CK):
                    ps = small_ps.tile([P, P], F32, tag="sp")
                    nc.tensor.transpose(ps[:], xin[:, c * P : (c + 1) * P], ident[:])
                    nc.vector.tensor_copy(
                        out=xT(c)[:, tt * P : (tt + 1) * P], in_=ps[:]
                    )

            # ---- v = (x w_v) + bias, bf16, with ones column ----
            wv_sb = rotp.tile([P, CK, HC], F32R, tag="rot")
            nc.sync.dma_start(wv_sb[:], wv_d[:])
            for tt in range(TT):
                ps = mm_ps.tile([P, 512], F32, tag="mm")
                for ck in range(CK):
                    nc.tensor.matmul(
                        ps[:],
                        xT(ck)[:, tt * P : (tt + 1) * P],
                        wv_sb[:, ck, :],
                        start=(ck == 0),
                        stop=(ck == CK - 1),
                    )
                nc.vector.tensor_add(
                    out=v_ext[:, tt, :, 0:D],
                    in0=ps[:].rearrange("p (h d) -> p h d", d=D),
                    in1=bv_sb[:].rearrange("p (h d) -> p h d", d=D),
                )

            # ---- q^T, k^T (bf16) = (w^T x^T) + bias ----
            for w_d, b_sb, dstT in ((wq_d, bq_sb, qT), (wk_d, bk_sb, kT)):
                for j in range(NP):
                    wj = wqkp.tile([P, CK, P], F32R, tag="wqk")
                    nc.sync.dma_start(wj[:], w_d[:, j])
                    for u in range(T // 512):
                        ps = mm_ps.tile([P, 512], F32, tag="mm")
                        for ck in range(CK):
                            nc.tensor.matmul(
                                ps[:],
                                wj[:, ck, :],
                                xT(ck)[:, u * 512 : (u + 1) * 512],
                                start=(ck == 0),
                                stop=(ck == CK - 1),
                            )
                        nc.vector.tensor_add(
                            out=dstT[:, j, u * 512 : (u + 1) * 512],
                            in0=ps[:],
                            in1=b_sb[:, j : j + 1].to_broadcast((P, 512)),
                        )

            # ---- attention per head pair (row-packed qk) ----
            for j in range(NP):
                for u in range(2):  # t in [1024u, 1024u+1024)
                    n_i = 8 * (u + 1)  # s-chunks needed for this half
                    p_a = rotp.tile([P, TT, 1024], BF16, tag="rot")
                    p_b = rotp.tile([P, TT, 1024], BF16, tag="rot")
                    for i in range(n_i):
                        ps2 = [
                            score_ps.tile(
                                [P, 1024], F32, tag="score", name=f"score{hh}"
                            )
                            for hh in range(2)
                        ]
                        for hh in range(2):  # heads 2j, 2j+1 run concurrently
                            hb = hh * D
                            for jj in range(2 * u, 2 * u + 2):
                                if jj < i // 4:
                                    continue
                                c0 = (jj - 2 * u) * 512
                                nc.tensor.matmul(
                                    ps2[hh][:, c0 : c0 + 512],
                                    kT[hb : hb + D, j, i * P : (i + 1) * P],
                                    qT[hb : hb + D, j, jj * 512 : (jj + 1) * 512],
                                    start=True,
                                    stop=True,
                                )
                        for hh, p_sb in ((0, p_a), (1, p_b)):
                            if i // 8 == u:
                                d0 = i * P - 1024 * u
                                nc.vector.tensor_add(
                                    out=ps2[hh][:, d0 : d0 + P],
                                    in0=ps2[hh][:, d0 : d0 + P],
                                    in1=dmask[:],
                                )
                            c0 = max(0, i * P - 1024 * u)
                            nc.scalar.activation(
                                out=p_sb[:, i, c0:1024],
                                in_=ps2[hh][:, c0:1024],
                                func=mybir.ActivationFunctionType.Exp,
                                scale=SCALE,
                            )
                    for tl in range(8):
                        t_chunk = 8 * u + tl
                        y_pair = ypairp.tile([P, P], BF16, tag="yp")
                        for hh, p_sb in ((0, p_a), (1, p_b)):
                            h = 2 * j + hh
                            ps_av = small_ps.tile([P, D + 1], F32, tag="sp")
                            for i in range(t_chunk + 1):
                                nc.tensor.matmul(
                                    ps_av[:],
                                    p_sb[:, i, tl * P : (tl + 1) * P],
                                    v_ext[:, i, h, :],
                                    start=(i == 0),
                                    stop=(i == t_chunk),
                                )
                            recip = workp.tile([P, 1], F32, tag="recip")
                            nc.vector.reciprocal(recip[:], ps_av[:, D : D + 1])
                            nc.vector.tensor_mul(
                                out=y_pair[:, hh * D : (hh + 1) * D],
                                in0=ps_av[:, 0:D],
                                in1=recip[:, 0:1].to_broadcast((P, D)),
                            )
                        # transpose y block into yT
                        ps = small_ps.tile([P, P], BF16, tag="sp")
                        nc.tensor.transpose(ps[:], y_pair[:], ident_bf[:])
                        nc.vector.tensor_copy(
                            out=yT[:, j, t_chunk * P : (t_chunk + 1) * P],
                            in_=ps[:],
                        )

            # ---- c_proj + bias -> z_dram; chunked pair ReduceScatter ----
            wp_sb = rotp.tile([P, HC // P, C], BF16, tag="rot")
            nc.sync.dma_start(wp_sb[:], wp_d[:])
            z_dram = dramp.tile([T, C], F32)
            rs_out = dramp.tile([T // 2, C], F32)
            tt_per_chunk = TT // RS_CHUNKS
            rows = T // RS_CHUNKS  # 512
            half = rows // 2  # 256
            for rc in range(RS_CHUNKS):
                for tt in range(rc * tt_per_chunk, (rc + 1) * tt_per_chunk):
                    for n in range(C // 512):
                        ps = mm_ps.tile([P, 512], F32, tag="mm")
                        for c in range(HC // P):
                            nc.tensor.matmul(
                                ps[:],
                                yT[:, c, tt * P : (tt + 1) * P],
                                wp_sb[:, c, n * 512 : (n + 1) * 512],
                                start=(c == 0),
                                stop=(c == HC // P - 1),
                            )
                        z_sb = zoutp.tile([P, 512], F32, tag="z")
                        nc.vector.tensor_add(
                            out=z_sb[:],
                            in0=ps[:],
                            in1=bp_sb[:, n * 512 : (n + 1) * 512],
                        )
                        nc.sync.dma_start(
                            z_dram[tt * P : (tt + 1) * P, n * 512 : (n + 1) * 512],
                            z_sb[:],
                        )
                nc.gpsimd.collective_compute(
                    "ReduceScatter",
                    mybir.AluOpType.add,
                    replica_groups=[[0, 1], [2, 3], [4, 5], [6, 7]],
                    ins=[z_dram[rc * rows : (rc + 1) * rows, :].opt()],
                    outs=[rs_out[rc * half : (rc + 1) * half, :].opt()],
                )
                nc.sync.dma_start(
                    out_d[rc * half : (rc + 1) * half, :],
                    rs_out[rc * half : (rc + 1) * half, :],
                )

    nc.compile()
    return nc


def _in_maps(inputs):
    x = np.ascontiguousarray(inputs["x"], dtype=np.float32)
    w_attn = np.asarray(inputs["w_attn"], dtype=np.float32)
    b_attn = np.asarray(inputs["b_attn"], dtype=np.float32)
    w_proj = np.asarray(inputs["w_proj"], dtype=np.float32)
    b_proj = np.asarray(inputs["b_proj"], dtype=np.float32)

    maps = []
    for core in range(N_CORES):
        b, g = core // 2, core % 2
        s = g * HC
        # [C, HC] -> [ki, j, ko, n] with c = ko*128+ki, qcol = j*128+n
        wq = w_attn[:, s : s + HC].reshape(CK, P, NP, P).transpose(1, 2, 0, 3)
        wk = w_attn[:, C + s : C + s + HC].reshape(CK, P, NP, P).transpose(1, 2, 0, 3)
        # [C, HC] -> [ki, ko, vcol]
        wv = (
            w_attn[:, 2 * C + s : 2 * C + s + HC].reshape(CK, P, HC).transpose(1, 0, 2)
        )
        # [HC, C] -> [ki, ko, co], bf16
        wp = (
            w_proj[s : s + HC, :]
            .reshape(HC // P, P, C)
            .transpose(1, 0, 2)
            .astype(ml_dtypes.bfloat16)
        )
        bq = b_attn[s : s + HC].reshape(NP, P).T
        bk = b_attn[C + s : C + s + HC].reshape(NP, P).T
        bv = np.broadcast_to(b_attn[2 * C + s : 2 * C + s + HC], (P, HC))
        bp = (
            np.broadcast_to(b_proj, (P, C))
            if g == 0
            else np.zeros((P, C), np.float32)
        )
        maps.append(
            {
                "x": x[b],
                "wq": np.ascontiguousarray(wq),
                "wk": np.ascontiguousarray(wk),
                "wv": np.ascontiguousarray(wv),
                "wp": np.ascontiguousarray(wp),
                "bq": np.ascontiguousarray(bq),
                "bk": np.ascontiguousarray(bk),
                "bv": np.ascontiguousarray(bv),
                "bp": np.ascontiguousarray(bp),
            }
        )
    return maps


def _run(inputs, trace=False, trace_cores=None):
    if "nc" not in _CACHE:
        _CACHE["nc"] = _build_nc()
    nc = _CACHE["nc"]
    res = run_bass_kernel_spmd(
        nc,
        _in_maps(inputs),
        list(range(N_CORES)),
        trace=trace,
        trace_cores=trace_cores,
    )
    # chunked RS ownership: even core holds rows [512c, 512c+256),
    # odd core holds rows [512c+256, 512c+512), for c = 0..3
    out = np.empty((B, T, C), np.float32)
    rows = T // RS_CHUNKS
    half = rows // 2
    for b in range(B):
        ev = res.results[2 * b]["out"]
        od = res.results[2 * b + 1]["out"]
        for rc in range(RS_CHUNKS):
            out[b, rc * rows : rc * rows + half] = ev[rc * half : (rc + 1) * half]
            out[b, rc * rows + half : (rc + 1) * rows] = od[
                rc * half : (rc + 1) * half
            ]
    return out, res


def kernel(**inputs):
    out, _ = _run(inputs)
    return out


# revision 8
# speedup vs baseline: 1.5080x; 1.1711x over previous
"""Causal self-attention (B=4, T=2048, C=1024, H=16) on 8 Trainium2 cores.

Sharding: core c -> batch b = c//2, head-group g = c%2 (8 heads each,
tensor-parallel). QKV + attention + c_proj computed per core on its head
slice; partial c_proj outputs of a (b) pair are summed with chunked
on-device ReduceScatters over the T dimension; host reassembles.

Self-contained: only imports concourse (installed library) + numpy.
"""

import ml_dtypes
import numpy as np

import concourse.mybir as mybir
import concourse.tile as tile
from concourse import bacc
from concourse.bass_utils import run_bass_kernel_spmd
from concourse.masks import make_identity

B, T, C = 4, 2048, 1024
H_TOTAL, D = 16, 64
N_CORES = 8
HL = H_TOTAL // 2  # local heads per core (8)
HC = HL * D  # local head cols (512)
NP = HL // 2  # head pairs (4)
P = 128
TT = T // P  # 16 t-chunks of 128
CK = C // P  # 8 contraction chunks for qkv
RS_CHUNKS = 4
F32 = mybir.dt.float32
F32R = mybir.dt.float32r
BF16 = mybir.dt.bfloat16
MASK_VAL = -480.0  # -60 after the 1/8 attention scale; exp(-60) ~ 0
SCALE = 1.0 / 8.0  # 1/sqrt(D)

_CACHE = {}


def _build_nc():
    nc = bacc.Bacc("TRN2", target_bir_lowering=False, debug=False, num_devices=N_CORES)

    x_d = nc.dram_tensor("x", [T, C], F32, kind="ExternalInput")
    # weights pre-laid-out on host for contiguous DMA
    wq_d = nc.dram_tensor("wq", [P, NP, CK, P], F32R, kind="ExternalInput")
    wk_d = nc.dram_tensor("wk", [P, NP, CK, P], F32R, kind="ExternalInput")
    wv_d = nc.dram_tensor("wv", [P, CK, HC], F32R, kind="ExternalInput")
    bq_d = nc.dram_tensor("bq", [P, NP], F32, kind="ExternalInput")
    bk_d = nc.dram_tensor("bk", [P, NP], F32, kind="ExternalInput")
    bv_d = nc.dram_tensor("bv", [P, HC], F32, kind="ExternalInput")
    wp_d = nc.dram_tensor("wp", [P, HC // P, C], BF16, kind="ExternalInput")
    bp_d = nc.dram_tensor("bp", [P, C], F32, kind="ExternalInput")
    out_d = nc.dram_tensor("out", [T // 2, C], BF16, kind="ExternalOutput")

    with tile.TileContext(nc) as tc:
        with (
            tc.tile_pool(name="const", bufs=1) as constp,
            tc.tile_pool(name="big", bufs=1) as bigp,
            tc.tile_pool(name="rot", bufs=3) as rotp,
            tc.tile_pool(name="xin", bufs=2) as xinp,
            tc.tile_pool(name="wqk", bufs=2) as wqkp,
            tc.tile_pool(name="wpp", bufs=1) as wppp,
            tc.tile_pool(name="ypair", bufs=4) as ypairp,
            tc.tile_pool(name="work", bufs=4) as workp,
            tc.tile_pool(name="zout", bufs=2) as zoutp,
            tc.tile_pool(name="score_ps", bufs=2, space="PSUM") as score_ps,
            tc.tile_pool(name="small_ps", bufs=2, space="PSUM") as small_ps,
            tc.tile_pool(name="mm_ps", bufs=2, space="PSUM") as mm_ps,
            tc.tile_pool(name="dram", bufs=1, space="DRAM") as dramp,
        ):
            # ---- constants ----
            ident = constp.tile([P, P], F32)
            make_identity(nc, ident)
            ident_bf = constp.tile([P, P], BF16)
            nc.vector.tensor_copy(out=ident_bf[:], in_=ident[:])
            # additive causal mask for the diagonal 128x128 block:
            # mask[s, u] = 0 where u >= s else MASK_VAL
            dmask = constp.tile([P, P], F32)
            nc.gpsimd.memset(dmask, 0.0)
            nc.gpsimd.affine_select(
                out=dmask,
                in_=dmask,
                compare_op=mybir.AluOpType.is_ge,
                fill=MASK_VAL,
                base=0,
                pattern=[[1, P]],
                channel_multiplier=-1,
            )
            bq_sb = constp.tile([P, NP], F32)
            nc.sync.dma_start(bq_sb[:], bq_d[:])
            bk_sb = constp.tile([P, NP], F32)
            nc.sync.dma_start(bk_sb[:], bk_d[:])
            bv_sb = constp.tile([P, HC], F32)
            nc.sync.dma_start(bv_sb[:], bv_d[:])
            bp_sb = constp.tile([P, C], F32)
            nc.sync.dma_start(bp_sb[:], bp_d[:])

            # ---- persistent activations ----
            qT = bigp.tile([P, NP, T], BF16)  # q^T [qcol, t]
            kT = bigp.tile([P, NP, T], BF16)  # k^T [kcol, t]
            v_ext = bigp.tile([P, TT, HL, D + 1], BF16)  # v with ones col
            yT = bigp.tile([P, NP, T], BF16)  # y^T [ci, t]
            # rotating 32KB slots: xT halves + wv -> p buffers -> wp
            xT_a = rotp.tile([P, CK // 2, T], F32R, tag="rot")
            xT_b = rotp.tile([P, CK // 2, T], F32R, tag="rot")

            def xT(ck):
                return xT_a[:, ck] if ck < CK // 2 else xT_b[:, ck - CK // 2]

            nc.vector.memset(v_ext[:, :, :, D : D + 1], 1.0)

            # ---- transpose x: [T, C] -> xT [C, T] ----
            for tt in range(TT):
                xin = xinp.tile([P, C], F32, tag="xin")
                nc.sync.dma_start(xin[:], x_d[tt * P : (tt + 1) * P, :])
                for c in range(CK):
                    ps = small_ps.tile([P, P], F32, tag="sp")
                    nc.tensor.transpose(ps[:], xin[:, c * P : (c + 1) * P], ident[:])
                    nc.vector.tensor_copy(
                        out=xT(c)[:, tt * P : (tt + 1) * P], in_=ps[:]
                    )

            # ---- v = (x w_v) + bias, bf16, with ones column ----
            wv_sb = rotp.tile([P, CK, HC], F32R, tag="rot")
            nc.sync.dma_start(wv_sb[:], wv_d[:])
            for tt in range(TT):
                ps = mm_ps.tile([P, 512], F32, tag="mm")
                for ck in range(CK):
                    nc.tensor.matmul(
                        ps[:],
                        xT(ck)[:, tt * P : (tt + 1) * P],
                        wv_sb[:, ck, :],
                        start=(ck == 0),
                        stop=(ck == CK - 1),
                    )
                nc.vector.tensor_add(
                    out=v_ext[:, tt, :, 0:D],
                    in0=ps[:].rearrange("p (h d) -> p h d", d=D),
                    in1=bv_sb[:].rearrange("p (h d) -> p h d", d=D),
                )

            # ---- attention (t-major, software-pipelined) + proj/RS ----
            wp_sb = wppp.tile([P, HC // P, C], BF16)
            nc.sync.dma_start(wp_sb[:], wp_d[:])
            z_dram = dramp.tile([T, C], BF16)
            rs_out = dramp.tile([T // 2, C], BF16)

            def qkproj(j):
                for w_d, b_sb, dstT in ((wq_d, bq_sb, qT), (wk_d, bk_sb, kT)):
                    wj = wqkp.tile([P, CK, P], F32R, tag="wqk", name=f"w{j}")
                    nc.sync.dma_start(wj[:], w_d[:, j])
                    for u in range(T // 512):
                        ps = mm_ps.tile([P, 512], F32, tag="mm", name="qk_ps")
                        for ck in range(CK):
                            nc.tensor.matmul(
                                ps[:],
                                wj[:, ck, :],
                                xT(ck)[:, u * 512 : (u + 1) * 512],
                                start=(ck == 0),
                                stop=(ck == CK - 1),
                            )
                        nc.vector.tensor_add(
                            out=dstT[:, j, u * 512 : (u + 1) * 512],
                            in0=ps[:],
                            in1=b_sb[:, j : j + 1].to_broadcast((P, 512)),
                        )

            def score_exp(j, u):
                n_i = 8 * (u + 1)
                p_a = rotp.tile([P, TT, 1024], BF16, tag="rot", name=f"pa{j}{u}")
                p_b = rotp.tile([P, TT, 1024], BF16, tag="rot", name=f"pb{j}{u}")
                for i in range(n_i):
                    ps2 = [
                        score_ps.tile([P, 1024], F32, tag="score", name=f"sc{hh}")
                        for hh in range(2)
                    ]
                    for hh in range(2):  # heads 2j, 2j+1 run concurrently
                        hb = hh * D
                        for jj in range(2 * u, 2 * u + 2):
                            if jj < i // 4:
                                continue
                            c0 = (jj - 2 * u) * 512
                            nc.tensor.matmul(
                                ps2[hh][:, c0 : c0 + 512],
                                kT[hb : hb + D, j, i * P : (i + 1) * P],
                                qT[hb : hb + D, j, jj * 512 : (jj + 1) * 512],
                                start=True,
                                stop=True,
                            )
                    for hh, p_sb in ((0, p_a), (1, p_b)):
                        if i // 8 == u:
                            d0 = i * P - 1024 * u
                            nc.vector.tensor_add(
                                out=ps2[hh][:, d0 : d0 + P],
                                in0=ps2[hh][:, d0 : d0 + P],
                                in1=dmask[:],
                            )
                        c0 = max(0, i * P - 1024 * u)
                        nc.scalar.activation(
                            out=p_sb[:, i, c0:1024],
                            in_=ps2[hh][:, c0:1024],
                            func=mybir.ActivationFunctionType.Exp,
                            scale=SCALE,
                        )
                return p_a, p_b

            def av(j, u, p_a, p_b):
                for tl in range(8):
                    t_chunk = 8 * u + tl
                    y_pair = ypairp.tile([P, P], BF16, tag="yp", name="y_pair")
                    for hh, p_sb in ((0, p_a), (1, p_b)):
                        h = 2 * j + hh
                        ps_av = small_ps.tile([P, D + 1], F32, tag="sp", name="av_ps")
                        for i in range(t_chunk + 1):
                            nc.tensor.matmul(
                                ps_av[:],
                                p_sb[:, i, tl * P : (tl + 1) * P],
                                v_ext[:, i, h, :],
                                start=(i == 0),
                                stop=(i == t_chunk),
                            )
                        recip = workp.tile([P, 1], F32, tag="recip", name="recip")
                        nc.vector.reciprocal(recip[:], ps_av[:, D : D + 1])
                        nc.vector.tensor_mul(
                            out=y_pair[:, hh * D : (hh + 1) * D],
                            in0=ps_av[:, 0:D],
                            in1=recip[:, 0:1].to_broadcast((P, D)),
                        )
                    ps = small_ps.tile([P, P], BF16, tag="sp", name="yt_ps")
                    nc.tensor.transpose(ps[:], y_pair[:], ident_bf[:])
                    nc.vector.tensor_copy(
                        out=yT[:, j, t_chunk * P : (t_chunk + 1) * P],
                        in_=ps[:],
                    )

            rows = T // RS_CHUNKS  # 512
            half = rows // 2  # 256

            def proj_rs(rc):
                tt_per_chunk = TT // RS_CHUNKS
                for tt in range(rc * tt_per_chunk, (rc + 1) * tt_per_chunk):
                    for n in range(C // 512):
                        ps = mm_ps.tile([P, 512], F32, tag="mm", name="pj_ps")
                        for c in range(HC // P):
                            nc.tensor.matmul(
                                ps[:],
                                yT[:, c, tt * P : (tt + 1) * P],
                                wp_sb[:, c, n * 512 : (n + 1) * 512],
                                start=(c == 0),
                                stop=(c == HC // P - 1),
                            )
                        z_sb = zoutp.tile([P, 512], BF16, tag="z", name="z_sb")
                        nc.vector.tensor_add(
                            out=z_sb[:],
                            in0=ps[:],
                            in1=bp_sb[:, n * 512 : (n + 1) * 512],
                        )
                        nc.sync.dma_start(
                            z_dram[tt * P : (tt + 1) * P, n * 512 : (n + 1) * 512],
                            z_sb[:],
                        )
                nc.gpsimd.collective_compute(
                    "ReduceScatter",
                    mybir.AluOpType.add,
                    replica_groups=[[0, 1], [2, 3], [4, 5], [6, 7]],
                    ins=[z_dram[rc * rows : (rc + 1) * rows, :].opt()],
                    outs=[rs_out[rc * half : (rc + 1) * half, :].opt()],
                )
                nc.sync.dma_start(
                    out_d[rc * half : (rc + 1) * half, :],
                    rs_out[rc * half : (rc + 1) * half, :],
                )

            units = [(j, u) for u in range(2) for j in range(NP)]
            qkproj(0)
            prev = None
            for n, (j, u) in enumerate(units):
                ps_pair = score_exp(j, u)
                if n + 1 < len(units) and units[n + 1][1] == 0:
                    qkproj(units[n + 1][0])
                elif n + 1 == NP + 1:  # just entered u=1: u=0 rows done soon
                    pass
                if prev is not None:
                    av(*prev)
                    if prev[:2] == (NP - 1, 0):  # all u=0 y rows written
                        proj_rs(0)
                        proj_rs(1)
                prev = (j, u, *ps_pair)
            av(*prev)
            proj_rs(2)
            proj_rs(3)

    nc.compile()
    return nc


def _in_maps(inputs):
    x = np.ascontiguousarray(inputs["x"], dtype=np.float32)
    w_attn = np.asarray(inputs["w_attn"], dtype=np.float32)
    b_attn = np.asarray(inputs["b_attn"], dtype=np.float32)
    w_proj = np.asarray(inputs["w_proj"], dtype=np.float32)
    b_proj = np.asarray(inputs["b_proj"], dtype=np.float32)

    maps = []
    for core in range(N_CORES):
        b, g = core // 2, core % 2
        s = g * HC
        # [C, HC] -> [ki, j, ko, n] with c = ko*128+ki, qcol = j*128+n
        wq = w_attn[:, s : s + HC].reshape(CK, P, NP, P).transpose(1, 2, 0, 3)
        wk = w_attn[:, C + s : C + s + HC].reshape(CK, P, NP, P).transpose(1, 2, 0, 3)
        # [C, HC] -> [ki, ko, vcol]
        wv = (
            w_attn[:, 2 * C + s : 2 * C + s + HC].reshape(CK, P, HC).transpose(1, 0, 2)
        )
        # [HC, C] -> [ki, ko, co], bf16
        wp = (
            w_proj[s : s + HC, :]
            .reshape(HC // P, P, C)
            .transpose(1, 0, 2)
            .astype(ml_dtypes.bfloat16)
        )
        bq = b_attn[s : s + HC].reshape(NP, P).T
        bk = b_attn[C + s : C + s + HC].reshape(NP, P).T
        bv = np.broadcast_to(b_attn[2 * C + s : 2 * C + s + HC], (P, HC))
        bp = (
            np.broadcast_to(b_proj, (P, C))
            if g == 0
            else np.zeros((P, C), np.float32)
        )
        maps.append(
            {
                "x": x[b],
                "wq": np.ascontiguousarray(wq),
                "wk": np.ascontiguousarray(wk),
                "wv": np.ascontiguousarray(wv),
                "wp": np.ascontiguousarray(wp),
                "bq": np.ascontiguousarray(bq),
                "bk": np.ascontiguousarray(bk),
                "bv": np.ascontiguousarray(bv),
                "bp": np.ascontiguousarray(bp),
            }
        )
    return maps


def _run(inputs, trace=False, trace_cores=None):
    if "nc" not in _CACHE:
        _CACHE["nc"] = _build_nc()
    nc = _CACHE["nc"]
    res = run_bass_kernel_spmd(
        nc,
        _in_maps(inputs),
        list(range(N_CORES)),
        trace=trace,
        trace_cores=trace_cores,
    )
    # chunked RS ownership: even core holds rows [512c, 512c+256),
    # odd core holds rows [512c+256, 512c+512), for c = 0..3
    out = np.empty((B, T, C), np.float32)
    rows = T // RS_CHUNKS
    half = rows // 2
    for b in range(B):
        ev = res.results[2 * b]["out"].astype(np.float32)
        od = res.results[2 * b + 1]["out"].astype(np.float32)
        for rc in range(RS_CHUNKS):
            out[b, rc * rows : rc * rows + half] = ev[rc * half : (rc + 1) * half]
            out[b, rc * rows + half : (rc + 1) * rows] = od[
                rc * half : (rc + 1) * half
            ]
    return out, res


def kernel(**inputs):
    out, _ = _run(inputs)
    return out


# revision 9
# speedup vs baseline: 1.5813x; 1.0485x over previous
"""Causal self-attention (B=4, T=2048, C=1024, H=16) on 8 Trainium2 cores.

Sharding: core c -> batch b = c//2, head-group g = c%2 (8 heads each,
tensor-parallel). QKV + attention + c_proj computed per core on its head
slice; partial c_proj outputs of a (b) pair are summed with chunked
on-device ReduceScatters over the T dimension; host reassembles.

Self-contained: only imports concourse (installed library) + numpy.
"""

import ml_dtypes
import numpy as np

import concourse.mybir as mybir
import concourse.tile as tile
from concourse import bacc
from concourse.bass_utils import run_bass_kernel_spmd
from concourse.masks import make_identity

B, T, C = 4, 2048, 1024
H_TOTAL, D = 16, 64
N_CORES = 8
HL = H_TOTAL // 2  # local heads per core (8)
HC = HL * D  # local head cols (512)
NP = HL // 2  # head pairs (4)
P = 128
TT = T // P  # 16 t-chunks of 128
CK = C // P  # 8 contraction chunks for qkv
RS_CHUNKS = 4
F32 = mybir.dt.float32
F32R = mybir.dt.float32r
BF16 = mybir.dt.bfloat16
MASK_VAL = -480.0  # -60 after the 1/8 attention scale; exp(-60) ~ 0
SCALE = 1.0 / 8.0  # 1/sqrt(D)

_CACHE = {}


def _build_nc():
    nc = bacc.Bacc("TRN2", target_bir_lowering=False, debug=False, num_devices=N_CORES)

    x_d = nc.dram_tensor("x", [T, C], F32, kind="ExternalInput")
    # weights pre-laid-out on host for contiguous DMA
    wq_d = nc.dram_tensor("wq", [P, NP, CK, P], BF16, kind="ExternalInput")
    wk_d = nc.dram_tensor("wk", [P, NP, CK, P], BF16, kind="ExternalInput")
    wv_d = nc.dram_tensor("wv", [P, CK, HC], BF16, kind="ExternalInput")
    bq_d = nc.dram_tensor("bq", [P, NP], F32, kind="ExternalInput")
    bk_d = nc.dram_tensor("bk", [P, NP], F32, kind="ExternalInput")
    bv_d = nc.dram_tensor("bv", [P, HC], F32, kind="ExternalInput")
    wp_d = nc.dram_tensor("wp", [P, HC // P, C], BF16, kind="ExternalInput")
    bp_d = nc.dram_tensor("bp", [P, C], F32, kind="ExternalInput")
    out_d = nc.dram_tensor("out", [T // 2, C], BF16, kind="ExternalOutput")

    with tile.TileContext(nc) as tc:
        with (
            tc.tile_pool(name="const", bufs=1) as constp,
            tc.tile_pool(name="big", bufs=1) as bigp,
            tc.tile_pool(name="rot", bufs=3) as rotp,
            tc.tile_pool(name="xin", bufs=2) as xinp,
            tc.tile_pool(name="wqk", bufs=2) as wqkp,
            tc.tile_pool(name="wpp", bufs=1) as wppp,
            tc.tile_pool(name="wvp", bufs=1) as wvp,
            tc.tile_pool(name="ypair", bufs=4) as ypairp,
            tc.tile_pool(name="work", bufs=4) as workp,
            tc.tile_pool(name="zout", bufs=2) as zoutp,
            tc.tile_pool(name="score_ps", bufs=2, space="PSUM") as score_ps,
            tc.tile_pool(name="small_ps", bufs=2, space="PSUM") as small_ps,
            tc.tile_pool(name="mm_ps", bufs=2, space="PSUM") as mm_ps,
            tc.tile_pool(name="dram", bufs=1, space="DRAM") as dramp,
        ):
            # ---- constants ----
            ident = constp.tile([P, P], F32)
            make_identity(nc, ident)
            ident_bf = constp.tile([P, P], BF16)
            nc.vector.tensor_copy(out=ident_bf[:], in_=ident[:])
            # additive causal mask for the diagonal 128x128 block:
            # mask[s, u] = 0 where u >= s else MASK_VAL
            dmask = constp.tile([P, P], F32)
            nc.gpsimd.memset(dmask, 0.0)
            nc.gpsimd.affine_select(
                out=dmask,
                in_=dmask,
                compare_op=mybir.AluOpType.is_ge,
                fill=MASK_VAL,
                base=0,
                pattern=[[1, P]],
                channel_multiplier=-1,
            )
            bq_sb = constp.tile([P, NP], F32)
            nc.sync.dma_start(bq_sb[:], bq_d[:])
            bk_sb = constp.tile([P, NP], F32)
            nc.sync.dma_start(bk_sb[:], bk_d[:])
            bv_sb = constp.tile([P, HC], F32)
            nc.sync.dma_start(bv_sb[:], bv_d[:])
            bp_sb = constp.tile([P, C], F32)
            nc.sync.dma_start(bp_sb[:], bp_d[:])

            # ---- persistent activations ----
            qT = bigp.tile([P, NP, T], BF16)  # q^T [qcol, t]
            kT = bigp.tile([P, NP, T], BF16)  # k^T [kcol, t]
            v_ext = bigp.tile([P, TT, HL, D + 1], BF16)  # v with ones col
            yT = bigp.tile([P, NP, T], BF16)  # y^T [ci, t]
            # rotating 32KB slots: xT -> p buffers
            xT_t = rotp.tile([P, CK, T], BF16, tag="rot")

            def xT(ck):
                return xT_t[:, ck]

            nc.vector.memset(v_ext[:, :, :, D : D + 1], 1.0)

            # ---- transpose x: [T, C] -> xT [C, T] ----
            for tt in range(TT):
                xin = xinp.tile([P, C], F32, tag="xin")
                nc.sync.dma_start(xin[:], x_d[tt * P : (tt + 1) * P, :])
                for c in range(CK):
                    ps = small_ps.tile([P, P], F32, tag="sp")
                    nc.tensor.transpose(ps[:], xin[:, c * P : (c + 1) * P], ident[:])
                    nc.vector.tensor_copy(
                        out=xT(c)[:, tt * P : (tt + 1) * P], in_=ps[:]
                    )

            # ---- v = (x w_v) + bias, bf16, with ones column ----
            wv_sb = wvp.tile([P, CK, HC], BF16)
            nc.sync.dma_start(wv_sb[:], wv_d[:])
            for tt in range(TT):
                ps = mm_ps.tile([P, 512], F32, tag="mm")
                for ck in range(CK):
                    nc.tensor.matmul(
                        ps[:],
                        xT(ck)[:, tt * P : (tt + 1) * P],
                        wv_sb[:, ck, :],
                        start=(ck == 0),
                        stop=(ck == CK - 1),
                    )
                nc.vector.tensor_add(
                    out=v_ext[:, tt, :, 0:D],
                    in0=ps[:].rearrange("p (h d) -> p h d", d=D),
                    in1=bv_sb[:].rearrange("p (h d) -> p h d", d=D),
                )

            # ---- attention (t-major, software-pipelined) + proj/RS ----
            wp_sb = wppp.tile([P, HC // P, C], BF16)
            nc.sync.dma_start(wp_sb[:], wp_d[:])
            z_dram = dramp.tile([T, C], BF16)
            rs_out = dramp.tile([T // 2, C], BF16)

            def qkproj(j):
                for w_d, b_sb, dstT in ((wq_d, bq_sb, qT), (wk_d, bk_sb, kT)):
                    wj = wqkp.tile([P, CK, P], BF16, tag="wqk", name=f"w{j}")
                    nc.sync.dma_start(wj[:], w_d[:, j])
                    for u in range(T // 512):
                        ps = mm_ps.tile([P, 512], F32, tag="mm", name="qk_ps")
                        for ck in range(CK):
                            nc.tensor.matmul(
                                ps[:],
                                wj[:, ck, :],
                                xT(ck)[:, u * 512 : (u + 1) * 512],
                                start=(ck == 0),
                                stop=(ck == CK - 1),
                            )
                        nc.vector.tensor_add(
                            out=dstT[:, j, u * 512 : (u + 1) * 512],
                            in0=ps[:],
                            in1=b_sb[:, j : j + 1].to_broadcast((P, 512)),
                        )

            def score_exp(j, u):
                n_i = 8 * (u + 1)
                p_a = rotp.tile([P, TT, 1024], BF16, tag="rot", name=f"pa{j}{u}")
                p_b = rotp.tile([P, TT, 1024], BF16, tag="rot", name=f"pb{j}{u}")
                for i in range(n_i):
                    ps2 = [
                        score_ps.tile([P, 1024], F32, tag="score", name=f"sc{hh}")
                        for hh in range(2)
                    ]
                    for hh in range(2):  # heads 2j, 2j+1 run concurrently
                        hb = hh * D
                        for jj in range(2 * u, 2 * u + 2):
                            if jj < i // 4:
                                continue
                            c0 = (jj - 2 * u) * 512
                            nc.tensor.matmul(
                                ps2[hh][:, c0 : c0 + 512],
                                kT[hb : hb + D, j, i * P : (i + 1) * P],
                                qT[hb : hb + D, j, jj * 512 : (jj + 1) * 512],
                                start=True,
                                stop=True,
                            )
                    for hh, p_sb in ((0, p_a), (1, p_b)):
                        if i // 8 == u:
                            d0 = i * P - 1024 * u
                            nc.vector.tensor_add(
                                out=ps2[hh][:, d0 : d0 + P],
                                in0=ps2[hh][:, d0 : d0 + P],
                                in1=dmask[:],
                            )
                        c0 = max(0, i * P - 1024 * u)
                        nc.scalar.activation(
                            out=p_sb[:, i, c0:1024],
                            in_=ps2[hh][:, c0:1024],
                            func=mybir.ActivationFunctionType.Exp,
                            scale=SCALE,
                        )
                return p_a, p_b

            def av(j, u, p_a, p_b):
                for tl in range(8):
                    t_chunk = 8 * u + tl
                    y_pair = ypairp.tile([P, P], BF16, tag="yp", name="y_pair")
                    for hh, p_sb in ((0, p_a), (1, p_b)):
                        h = 2 * j + hh
                        ps_av = small_ps.tile([P, D + 1], F32, tag="sp", name="av_ps")
                        for i in range(t_chunk + 1):
                            nc.tensor.matmul(
                                ps_av[:],
                                p_sb[:, i, tl * P : (tl + 1) * P],
                                v_ext[:, i, h, :],
                                start=(i == 0),
                                stop=(i == t_chunk),
                            )
                        recip = workp.tile([P, 1], F32, tag="recip", name="recip")
                        nc.vector.reciprocal(recip[:], ps_av[:, D : D + 1])
                        nc.vector.tensor_mul(
                            out=y_pair[:, hh * D : (hh + 1) * D],
                            in0=ps_av[:, 0:D],
                            in1=recip[:, 0:1].to_broadcast((P, D)),
                        )
                    ps = small_ps.tile([P, P], BF16, tag="sp", name="yt_ps")
                    nc.tensor.transpose(ps[:], y_pair[:], ident_bf[:])
                    nc.vector.tensor_copy(
                        out=yT[:, j, t_chunk * P : (t_chunk + 1) * P],
                        in_=ps[:],
                    )

            rows = T // RS_CHUNKS  # 512
            half = rows // 2  # 256

            def proj_rs(rc):
                tt_per_chunk = TT // RS_CHUNKS
                for tt in range(rc * tt_per_chunk, (rc + 1) * tt_per_chunk):
                    for n in range(C // 512):
                        ps = mm_ps.tile([P, 512], F32, tag="mm", name="pj_ps")
                        for c in range(HC // P):
                            nc.tensor.matmul(
                                ps[:],
                                yT[:, c, tt * P : (tt + 1) * P],
                                wp_sb[:, c, n * 512 : (n + 1) * 512],
                                start=(c == 0),
                                stop=(c == HC // P - 1),
                            )
                        z_sb = zoutp.tile([P, 512], BF16, tag="z", name="z_sb")
                        nc.vector.tensor_add(
                            out=z_sb[:],
                            in0=ps[:],
                            in1=bp_sb[:, n * 512 : (n + 1) * 512],
                        )
                        nc.sync.dma_start(
                            z_dram[tt * P : (tt + 1) * P, n * 512 : (n + 1) * 512],
                            z_sb[:],
                        )
                nc.gpsimd.collective_compute(
                    "ReduceScatter",
                    mybir.AluOpType.add,
                    replica_groups=[[0, 1], [2, 3], [4, 5], [6, 7]],
                    ins=[z_dram[rc * rows : (rc + 1) * rows, :].opt()],
                    outs=[rs_out[rc * half : (rc + 1) * half, :].opt()],
                )
                nc.sync.dma_start(
                    out_d[rc * half : (rc + 1) * half, :],
                    rs_out[rc * half : (rc + 1) * half, :],
                )

            for j in range(NP):
                qkproj(j)
            units = [(j, u) for u in range(2) for j in range(NP)]
            prev = None
            for n, (j, u) in enumerate(units):
                ps_pair = score_exp(j, u)
                if prev is not None:
                    av(*prev)
                    if prev[:2] == (NP - 1, 0):  # all u=0 y rows written
                        proj_rs(0)
                        proj_rs(1)
                prev = (j, u, *ps_pair)
            av(*prev)
            proj_rs(2)
            proj_rs(3)

    nc.compile()
    return nc


def _in_maps(inputs):
    x = np.ascontiguousarray(inputs["x"], dtype=np.float32)
    w_attn = np.asarray(inputs["w_attn"], dtype=np.float32)
    b_attn = np.asarray(inputs["b_attn"], dtype=np.float32)
    w_proj = np.asarray(inputs["w_proj"], dtype=np.float32)
    b_proj = np.asarray(inputs["b_proj"], dtype=np.float32)

    maps = []
    for core in range(N_CORES):
        b, g = core // 2, core % 2
        s = g * HC
        # [C, HC] -> [ki, j, ko, n] with c = ko*128+ki, qcol = j*128+n
        wq = (
            w_attn[:, s : s + HC]
            .reshape(CK, P, NP, P)
            .transpose(1, 2, 0, 3)
            .astype(ml_dtypes.bfloat16)
        )
        wk = (
            w_attn[:, C + s : C + s + HC]
            .reshape(CK, P, NP, P)
            .transpose(1, 2, 0, 3)
            .astype(ml_dtypes.bfloat16)
        )
        # [C, HC] -> [ki, ko, vcol]
        wv = (
            w_attn[:, 2 * C + s : 2 * C + s + HC]
            .reshape(CK, P, HC)
            .transpose(1, 0, 2)
            .astype(ml_dtypes.bfloat16)
        )
        # [HC, C] -> [ki, ko, co], bf16
        wp = (
            w_proj[s : s + HC, :]
            .reshape(HC // P, P, C)
            .transpose(1, 0, 2)
            .astype(ml_dtypes.bfloat16)
        )
        bq = b_attn[s : s + HC].reshape(NP, P).T
        bk = b_attn[C + s : C + s + HC].reshape(NP, P).T
        bv = np.broadcast_to(b_attn[2 * C + s : 2 * C + s + HC], (P, HC))
        bp = (
            np.broadcast_to(b_proj, (P, C))
            if g == 0
            else np.zeros((P, C), np.float32)
        )
        maps.append(
            {
                "x": x[b],
                "wq": np.ascontiguousarray(wq),
                "wk": np.ascontiguousarray(wk),
                "wv": np.ascontiguousarray(wv),
                "wp": np.ascontiguousarray(wp),
                "bq": np.ascontiguousarray(bq),
                "bk": np.ascontiguousarray(bk),
                "bv": np.ascontiguousarray(bv),
                "bp": np.ascontiguousarray(bp),
            }
        )
    return maps


def _run(inputs, trace=False, trace_cores=None):
    if "nc" not in _CACHE:
        _CACHE["nc"] = _build_nc()
    nc = _CACHE["nc"]
    res = run_bass_kernel_spmd(
        nc,
        _in_maps(inputs),
        list(range(N_CORES)),
        trace=trace,
        trace_cores=trace_cores,
    )
    # chunked RS ownership: even core holds rows [512c, 512c+256),
    # odd core holds rows [512c+256, 512c+512), for c = 0..3
    out = np.empty((B, T, C), np.float32)
    rows = T // RS_CHUNKS
    half = rows // 2
    for b in range(B):
        ev = res.results[2 * b]["out"].astype(np.float32)
        od = res.results[2 * b + 1]["out"].astype(np.float32)
        for rc in range(RS_CHUNKS):
            out[b, rc * rows : rc * rows + half] = ev[rc * half : (rc + 1) * half]
            out[b, rc * rows + half : (rc + 1) * rows] = od[
                rc * half : (rc + 1) * half
            ]
    return out, res


def kernel(**inputs):
    out, _ = _run(inputs)
    return out
